# revision 1
# baseline (speedup 1.0000x reference)
"""Multi-head attention (B=2, S=2048, D=1024, H=16) on 8 trn2 NeuronCores.

Sharding: core c -> batch b = c//4, head-group g = c%4 (4 heads each).
Tensor-parallel on heads: each core projects q/k/v for its 4 heads only
(column-sharded W_q/W_k/W_v), runs full-sequence attention for those heads,
all-gathers the per-head attention outputs within its 4-core batch group,
and computes a 256-column slice of the final W_o projection. The host
reassembles the [2, 2048, 1024] output from the 8 per-core [256, 2048]
(transposed) slices.

Key structure (269-281us vs the 297us predecessor):
- All inputs are host-pre-swizzled so every DMA is a few large contiguous
  descriptors: X as [128, e-tile, S] (loaded as four 8KB-per-partition
  e-pair DMAs each), W as [128, e-tile, 256], biases packed into one
  [128, 6] tensor. X loads run on the sync hwdge ring, W on the scalar
  ring, in parallel. X order v -> k -> q with per-tensor buffers, so the
  projections chase the DMA stream by contraction tile and attention
  starts right after the q eviction (~70us, down from ~89us).
- V is projected directly into natural [s, h, d(+ones)] layout (X^T
  e-tiles as the stationary operand) -- no PE transposes, no identity.
  Key tiles 0-7 accumulate by-e as Xv streams; 8-15 from resident xv.
- q-scale 1/8 is folded into Wq on the host; the v-bias is folded into
  bo_eff = bo + Wo @ bv (softmax rows sum to 1), so v needs no bias.
- Projection PSUM evictions run on the DVE (tensor_scalar_add bias
  fusion): the scalar engine does nothing but the exp stream, which is
  the hard floor of the kernel (16.8M exps/core ~= 142us; measured
  stream ~151us with ~11us of pipeline gaps).
- Attention: each qT/kT tile holds a head PAIR in partition halves (the
  two K=64 score matmuls occupy different PE row-groups and overlap),
  one exp covers both heads, softmax sums ride the ones-column of the
  PV matmul, normalization (reciprocal + shuffle broadcast + multiply)
  runs on DVE/DMA only.
- A tiny warm-up AllGather fires at kernel start: the first collective
  pays ~60us of one-time ncfw latency, absorbed while the projections
  run. The real all-gather is split into 8 per-(pair, quarter) 128KB
  pieces fired as each quarter finishes; their ~12-18us services hide
  under the exp stream. Gathered pieces are prefetched to SBUF on the
  GPSIMD queue (which the blocking collectives already park, so the
  AG-dependent DMA triggers never stall another engine -- putting them
  on the scalar queue stalls the exp stream itself).
- The O projection runs as a tail on all 8 freed PSUM banks, pair-0
  e-tiles accumulated first so only the final AG piece gates the last
  accumulation step.
"""

import os

import ml_dtypes
import numpy as np

import concourse.bacc as bacc
import concourse.mybir as mybir
import concourse.tile as tile
from concourse import bass_utils

B, S, D, H = 2, 2048, 1024, 16
Dh = D // H  # 64
N_CORES = 8
HL = H // 4  # heads per core (4)
DL = HL * Dh  # local head dims (256)
P = 128
E_TILES = D // P  # 8
KT = S // P  # 16 key tiles
QQ = 512  # s_q quarter block

f32 = mybir.dt.float32
bf16 = mybir.dt.bfloat16
AF = mybir.ActivationFunctionType

TRACE = False  # test harness sets kernel.TRACE = True for profiling


def _build():
    nc = bacc.Bacc("TRN2", target_bir_lowering=False, debug=False,
                   num_devices=N_CORES)

    XvS = nc.dram_tensor("XvS", [P, E_TILES, S], bf16,
                         kind="ExternalInput").ap()
    XkS = nc.dram_tensor("XkS", [P, E_TILES, S], bf16,
                         kind="ExternalInput").ap()
    XqS = nc.dram_tensor("XqS", [P, E_TILES, S], bf16,
                         kind="ExternalInput").ap()
    WvS = nc.dram_tensor("WvS", [P, E_TILES, DL], bf16,
                         kind="ExternalInput").ap()
    WkS = nc.dram_tensor("WkS", [P, E_TILES, DL], bf16,
                         kind="ExternalInput").ap()
    WqS = nc.dram_tensor("WqS", [P, E_TILES, DL], bf16,
                         kind="ExternalInput").ap()
    WoS = nc.dram_tensor("WoS", [P, E_TILES, DL], bf16,
                         kind="ExternalInput").ap()
    bqko = nc.dram_tensor("bqko", [P, 6], f32, kind="ExternalInput").ap()
    out = nc.dram_tensor("out", [DL, S], f32, kind="ExternalOutput").ap()

    with tile.TileContext(nc) as tc:
        with (
            tc.tile_pool(name="const", bufs=1) as cp,
            tc.tile_pool(name="qk", bufs=1) as qkp,
            tc.tile_pool(name="vt", bufs=1) as vtp,
            tc.tile_pool(name="dram", bufs=1, space="DRAM") as dram,
        ):
            # ---- constants / weights (scalar hwdge ring) ----
            wv = cp.tile([P, E_TILES, DL], bf16, tag="wv")
            wk = cp.tile([P, E_TILES, DL], bf16, tag="wk")
            wq = cp.tile([P, E_TILES, DL], bf16, tag="wq")
            wo = cp.tile([P, E_TILES, DL], bf16, tag="wo")
            bias_c = cp.tile([P, 6], f32, tag="bias")
            bq_c = bias_c[:, 0:2]
            bk_c = bias_c[:, 2:4]
            bo_c = bias_c[:, 4:6]

            nc.scalar.dma_start(wv[:], WvS)
            nc.scalar.dma_start(bias_c[:], bqko)
            nc.scalar.dma_start(wk[:], WkS)
            nc.scalar.dma_start(wq[:], WqS)
            nc.scalar.dma_start(wo[:], WoS)

            # preload the exp table while DMAs stream
            warm = cp.tile([P, 1], f32, tag="warm")
            nc.gpsimd.memset(warm[:], 0.0)
            nc.scalar.activation(warm[:], warm[:], AF.Exp)

            # warm-up collective: absorbs the ~30us first-collective ncfw
            # setup cost while the projections run (gpsimd queue is idle)
            wagi = dram.tile([P, 4], bf16, tag="wagi", name="wagi")
            wago = dram.tile([4 * P, 4], bf16, tag="wago", name="wago")
            wsb = cp.tile([P, 4], bf16, tag="wsb")
            nc.vector.memset(wsb[:], 0.0)
            nc.gpsimd.dma_start(wagi[:], wsb[:])
            nc.gpsimd.collective_compute(
                "AllGather",
                mybir.AluOpType.bypass,
                replica_groups=[[0, 1, 2, 3], [4, 5, 6, 7]],
                ins=[wagi.opt()],
                outs=[wago.opt()],
            )

            v_sb = vtp.tile([P, KT, HL, Dh + 1], bf16)
            nc.vector.memset(v_sb[:, :, :, Dh], 1.0)
            kT = [
                qkp.tile([P, S], bf16, tag=f"kT{i}", name=f"kT{i}")
                for i in range(2)
            ]
            qT = [
                qkp.tile([P, S], bf16, tag=f"qT{i}", name=f"qT{i}")
                for i in range(2)
            ]

            # ---- projections (x pool scoped so its 96KB frees afterward)
            with tc.tile_pool(name="xs", bufs=1) as xsp:
                # X loads (sync hwdge ring): v first, then k, then q.
                xv = xsp.tile([P, E_TILES, S], bf16, tag="xv", name="xv")
                for ep in range(4):
                    nc.sync.dma_start(
                        xv[:, 2 * ep:2 * ep + 2, :],
                        XvS[:, 2 * ep:2 * ep + 2, :],
                    )
                xk = xsp.tile([P, E_TILES, S], bf16, tag="xk", name="xk")
                for ep in range(4):
                    nc.sync.dma_start(
                        xk[:, 2 * ep:2 * ep + 2, :],
                        XkS[:, 2 * ep:2 * ep + 2, :],
                    )
                xq = xsp.tile([P, E_TILES, S], bf16, tag="xq", name="xq")
                for ep in range(4):
                    nc.sync.dma_start(
                        xq[:, 2 * ep:2 * ep + 2, :],
                        XqS[:, 2 * ep:2 * ep + 2, :],
                    )

                # V -> natural [s, h, d(+ones)] layout, no bias. Key tiles
                # 0-7 accumulate by-e as Xv streams in; 8-15 afterward from
                # the resident xv.
                with tc.tile_pool(name="psv", bufs=1, space="PSUM") as psvp:
                    psv = [
                        psvp.tile([P, QQ], f32, tag=f"pv{i}",
                                  name=f"pv{i}", bufs=1)
                        for i in range(8)
                    ]
                    for e in range(E_TILES):
                        for kt in range(8):
                            nc.tensor.matmul(
                                psv[kt][:, 0:DL],
                                xv[:, e, kt * P:(kt + 1) * P],
                                wv[:, e, :],
                                start=(e == 0),
                                stop=(e == E_TILES - 1),
                            )
                    for kt in range(8):
                        nc.vector.tensor_copy(
                            v_sb[:, kt, :, 0:Dh],
                            psv[kt][:, 0:DL].rearrange(
                                "p (h d) -> p h d", h=HL
                            ),
                        )
                    psv2 = [
                        psvp.tile([P, QQ], f32, tag=f"pv{i}",
                                  name=f"pv2_{i}", bufs=1)
                        for i in range(8)
                    ]
                    for kt in range(8, KT):
                        for e in range(E_TILES):
                            nc.tensor.matmul(
                                psv2[kt - 8][:, 0:DL],
                                xv[:, e, kt * P:(kt + 1) * P],
                                wv[:, e, :],
                                start=(e == 0),
                                stop=(e == E_TILES - 1),
                            )
                        nc.vector.tensor_copy(
                            v_sb[:, kt, :, 0:Dh],
                            psv2[kt - 8][:, 0:DL].rearrange(
                                "p (h d) -> p h d", h=HL
                            ),
                        )

                # K / Q -> [d, s] layout (bias add on DVE)
                with tc.tile_pool(name="psp", bufs=1, space="PSUM") as pspp:
                    psk = [
                        pspp.tile([P, QQ], f32, tag=f"pp{i}", name=f"pk{i}",
                                  bufs=1)
                        for i in range(8)
                    ]
                    for e in range(E_TILES):
                        for dt in range(2):
                            for qb in range(4):
                                nc.tensor.matmul(
                                    psk[dt * 4 + qb][:],
                                    wk[:, e, dt * P:(dt + 1) * P],
                                    xk[:, e, qb * QQ:(qb + 1) * QQ],
                                    start=(e == 0),
                                    stop=(e == E_TILES - 1),
                                )
                    for dt in range(2):
                        for qb in range(4):
                            nc.vector.tensor_scalar_add(
                                kT[dt][:, qb * QQ:(qb + 1) * QQ],
                                psk[dt * 4 + qb][:],
                                bk_c[:, dt:dt + 1],
                            )
                    psq = [
                        pspp.tile([P, QQ], f32, tag=f"pp{i}", name=f"pq{i}",
                                  bufs=1)
                        for i in range(8)
                    ]
                    for e in range(E_TILES):
                        for dt in range(2):
                            for qb in range(4):
                                nc.tensor.matmul(
                                    psq[dt * 4 + qb][:],
                                    wq[:, e, dt * P:(dt + 1) * P],
                                    xq[:, e, qb * QQ:(qb + 1) * QQ],
                                    start=(e == 0),
                                    stop=(e == E_TILES - 1),
                                )
                    for dt in range(2):
                        for qb in range(4):
                            nc.vector.tensor_scalar_add(
                                qT[dt][:, qb * QQ:(qb + 1) * QQ],
                                psq[dt * 4 + qb][:],
                                bq_c[:, dt:dt + 1],
                            )

            # ---- attention: head-pairs x s_q-quarters x key-tiles ----
            # Each qT/kT tile holds a PAIR of heads in partition halves; the
            # pair's two K=64 score matmuls land in different PE row-groups
            # and run concurrently, one exp covers both heads' scores, and
            # softmax sums come free as the ones-column in the PV matmul.
            ag_in = [
                [
                    dram.tile([P, QQ], bf16, tag=f"agi{p}{qq}",
                              name=f"agi{p}{qq}")
                    for qq in range(4)
                ]
                for p in range(2)
            ]
            ag_out = [
                [
                    dram.tile([4 * P, QQ], bf16, tag=f"ago{p}{qq}",
                              name=f"ago{p}{qq}")
                    for qq in range(4)
                ]
                for p in range(2)
            ]

            with (
                tc.tile_pool(name="rhs", bufs=1) as rhp,
                tc.tile_pool(name="exp", bufs=3) as expp,
                tc.tile_pool(name="nrm", bufs=3) as nrmp,
                tc.tile_pool(name="ao", bufs=4) as aop,
                tc.tile_pool(name="res", bufs=4) as resp,
            ):
                rhs_sb = rhp.tile([P, 4, E_TILES, QQ], bf16)

                with (
                    tc.tile_pool(name="pss", bufs=2, space="PSUM") as pssp,
                    tc.tile_pool(name="pso", bufs=4, space="PSUM") as psop,
                ):
                    pso_cur = {}

                    def scores(u):
                        p, qq, kt = u
                        pss = pssp.tile(
                            [P, 2 * QQ], f32, tag="pss", name="pss"
                        )
                        for j in range(2):
                            nc.tensor.matmul(
                                pss[:, j * QQ:(j + 1) * QQ],
                                kT[p][
                                    64 * j:64 * j + Dh, kt * P:(kt + 1) * P
                                ],
                                qT[p][
                                    64 * j:64 * j + Dh,
                                    qq * QQ:(qq + 1) * QQ,
                                ],
                                start=True,
                                stop=True,
                            )
                        ex = expp.tile([P, 2 * QQ], bf16, tag="ex", name="ex")
                        nc.scalar.activation(ex[:], pss[:], AF.Exp)
                        return ex

                    def finish(p, qq, pso):
                        for j in range(2):
                            sums64 = nrmp.tile(
                                [Dh + 1, QQ], f32, tag="sm64", name="sm64"
                            )
                            nc.vector.tensor_copy(
                                sums64[Dh:Dh + 1, :], pso[j][Dh:Dh + 1, :]
                            )
                            sums = nrmp.tile([1, QQ], f32, tag="sm",
                                             name="sm")
                            nc.sync.dma_start(sums[:], sums64[Dh:Dh + 1, :])
                            recip = nrmp.tile([1, QQ], f32, tag="rc",
                                              name="rc")
                            nc.vector.reciprocal_approx_fast(
                                recip[:], sums[:]
                            )
                            rcb = nrmp.tile([64, QQ], f32, tag="rcb",
                                            name="rcb")
                            nc.sync.dma_start(rcb[0:1, :], recip[:])
                            nc.sync.dma_start(rcb[32:33, :], recip[:])
                            bc = nrmp.tile([64, QQ], f32, tag="bc", name="bc")
                            nc.vector.stream_shuffle(
                                bc[:], rcb[:], mask=[0] * 32
                            )
                            ao = aop.tile([Dh, QQ], bf16, tag="ao", name="ao")
                            nc.vector.tensor_mul(
                                ao[:], pso[j][0:Dh, :], bc[:]
                            )
                            nc.sync.dma_start(
                                ag_in[p][qq][64 * j:64 * j + Dh, :], ao[:]
                            )
                        nc.gpsimd.collective_compute(
                            "AllGather",
                            mybir.AluOpType.bypass,
                            replica_groups=[[0, 1, 2, 3], [4, 5, 6, 7]],
                            ins=[ag_in[p][qq].opt()],
                            outs=[ag_out[p][qq].opt()],
                        )
                        # prefetch on the gpsimd queue (already parked on CC)
                        for r in range(4):
                            nc.gpsimd.dma_start(
                                rhs_sb[:, qq, r * 2 + p, :],
                                ag_out[p][qq][r * P:(r + 1) * P, :],
                            )

                    def pv(u, ex):
                        p, qq, kt = u
                        if kt == 0:
                            pso_cur[(p, qq)] = [
                                psop.tile([Dh + 1, QQ], f32, tag="pso",
                                          name="pso")
                                for _ in range(2)
                            ]
                        pso = pso_cur[(p, qq)]
                        for j in range(2):
                            nc.tensor.matmul(
                                pso[j][:],
                                v_sb[:, kt, 2 * p + j, :],
                                ex[:, j * QQ:(j + 1) * QQ],
                                start=(kt == 0),
                                stop=(kt == KT - 1),
                            )
                        if kt == KT - 1:
                            finish(p, qq, pso)

                    units = [
                        (p, qq, kt)
                        for p in range(2)
                        for qq in range(4)
                        for kt in range(KT)
                    ]
                    ex_prev = scores(units[0])
                    for i in range(1, len(units)):
                        ex = scores(units[i])
                        pv(units[i - 1], ex_prev)
                        ex_prev = ex
                    pv(units[-1], ex_prev)

                # ---- O projection tail: out[dl, s] = WoS.T @ attn_full ---
                # pair-0 e-tiles first so only the last AG piece (pair 1 of
                # quarter 3) gates the final accumulation.
                eorder = (0, 2, 4, 6, 1, 3, 5, 7)
                with tc.tile_pool(name="pso2", bufs=1, space="PSUM") as pso2p:
                    pso2 = [
                        pso2p.tile([P, QQ], f32, tag=f"po{i}", name=f"po{i}",
                                   bufs=1)
                        for i in range(8)
                    ]
                    for qq in range(4):
                        for dt in range(2):
                            po = pso2[qq * 2 + dt]
                            for ei, e in enumerate(eorder):
                                nc.tensor.matmul(
                                    po[:],
                                    wo[:, e, dt * P:(dt + 1) * P],
                                    rhs_sb[:, qq, e, :],
                                    start=(ei == 0),
                                    stop=(ei == E_TILES - 1),
                                )
                            ot = resp.tile([P, QQ], f32, tag="ot", name="ot")
                            nc.scalar.activation(
                                ot[:],
                                po[:],
                                AF.Identity,
                                bias=bo_c[:, dt:dt + 1],
                                scale=1.0,
                            )
                            nc.sync.dma_start(
                                out[
                                    dt * P:(dt + 1) * P,
                                    qq * QQ:(qq + 1) * QQ,
                                ],
                                ot[:],
                            )

    nc.compile()
    return nc


def kernel(**inputs):
    Q = np.asarray(inputs["Q"], dtype=np.float32)
    K = np.asarray(inputs["K"], dtype=np.float32)
    V = np.asarray(inputs["V"], dtype=np.float32)
    Wq = np.asarray(inputs["Wq"], dtype=np.float32)
    Wk = np.asarray(inputs["Wk"], dtype=np.float32)
    Wv = np.asarray(inputs["Wv"], dtype=np.float32)
    Wo = np.asarray(inputs["Wo"], dtype=np.float32)
    bq = np.asarray(inputs["bq"], dtype=np.float32)
    bk = np.asarray(inputs["bk"], dtype=np.float32)
    bv = np.asarray(inputs["bv"], dtype=np.float32)
    bo = np.asarray(inputs["bo"], dtype=np.float32)

    nc = _build()

    def xswz(x):  # [S, D] -> [128, 8, S] (partition-contiguous e-tiles)
        return np.ascontiguousarray(
            x.T.reshape(E_TILES, P, S).transpose(1, 0, 2)
        ).astype(ml_dtypes.bfloat16)

    XT = {
        b: {
            "XvS": xswz(V[b]),
            "XkS": xswz(K[b]),
            "XqS": xswz(Q[b]),
        }
        for b in range(B)
    }

    # Heads within a group are laid out (0, 2, 1, 3) so each projected tile
    # holds a head pair spanning both PE row-group halves. W tensors are
    # pre-swizzled to the SBUF layout [128, e-tile, 256] for contiguous DMA.
    horder = (0, 2, 1, 3)

    def swz(w):  # [256 out-perm, 1024 in] -> [128, 8, 256]
        return np.ascontiguousarray(
            w.T.reshape(E_TILES, P, DL).transpose(1, 0, 2)
        ).astype(ml_dtypes.bfloat16)

    # Wo columns per e-tile follow the gathered attention rows:
    # e = (rank r, pair p); rows = heads horder[2p], horder[2p+1] of rank r.
    colperm = []
    for r in range(4):
        for p_ in range(2):
            for j in range(2):
                hl = horder[2 * p_ + j]
                colperm.extend(range(DL * r + 64 * hl, DL * r + 64 * hl + 64))
    colperm = np.array(colperm)

    Wslices = {}
    for g in range(4):
        idx = np.concatenate(
            [
                np.arange(DL * g + 64 * hl, DL * g + 64 * hl + 64)
                for hl in horder
            ]
        )
        rows = slice(DL * g, DL * (g + 1))
        Wslices[g] = {
            "WvS": swz(Wv[idx]),
            "WkS": swz(Wk[idx]),
            "WqS": swz(Wq[idx] * 0.125),
            "WoS": swz(Wo[rows][:, colperm]),
            "bqko": np.ascontiguousarray(
                np.stack(
                    [
                        bq[idx] * 0.125,
                        bk[idx],
                        bo[rows] + Wo[rows] @ bv,
                    ],
                    axis=0,
                )
                .reshape(3, 2, P)
                .transpose(2, 0, 1)
                .reshape(P, 6)
            ).astype(np.float32),
        }

    in_maps = []
    for c in range(N_CORES):
        b, g = c // 4, c % 4
        m = dict(XT[b])
        m.update(Wslices[g])
        in_maps.append(m)

    trace_cores = (
        list(range(N_CORES)) if os.environ.get("TRACE_ALL") else None
    )
    res = bass_utils.run_bass_kernel_spmd(
        nc, in_maps, core_ids=list(range(N_CORES)), trace=TRACE,
        trace_cores=trace_cores,
    )

    full = np.empty((B, S, D), dtype=np.float32)
    for c in range(N_CORES):
        b, g = c // 4, c % 4
        full[b, :, DL * g:DL * (g + 1)] = res.results[c]["out"].T
    if TRACE:
        kernel.last_result = res
    return full


kernel.last_result = None

